# revision 44
# baseline (speedup 1.0000x reference)
"""Trainium2 Bass kernel for nn_BottomUp (adding-doubling radiative transfer).

kernel(**inputs) takes FULL inputs a, r, t, s: [8192, 60, 48] fp32 and
returns (flux_up, flux_down, absorbed), each [8192, 59, 48] fp32.

Sharding: pure data parallel over examples E across 8 NeuronCores
(1024 examples per core, 8 chunks of 128 partitions each), no
communication.

Math per (e, c), layers l = 0..59 (layer 59 = surface):
  scan A (l = 59 -> 0), carry rs:
      tmp_l = rs_{l+1} * r_l
      id_l  = 1/(1 - tmp_l)
      rs_l  = (r_l + rs_{l+1} * t_l^2) * id_l
  bulk (l = 0..58), R = rs_{l+1}, ip = 1/(1+tmp):
      B1 = s+ * (2 - ip) + s * R * ip       (flux-up scan addend)
      w  = t * id                           (flux-up scan multiplier)
      C1 = (s + s+ * r) * id                (flux-down scan addend)
      tm = t * ip                           (flux-down scan multiplier)
  scan B (l = 58 -> 0): FU_l = w_{l+1} * FU_{l+1} + B1_l
  scan C (l = 0 -> 58): FD_l = tm_{l-1} * FD_{l-1} + C1_l
  absorbed = a * ((1 + t*R*ip) * FD + FU)

Scan A is parallelized layer-wise with a blocked halo: 10 blocks of 6
layers run in lockstep as wide [128, 480] ops; each block starts G=3
layers above its range with carry ~ r (the recurrence is a contraction,
so the halo error decays to ~1e-4).  A pseudo-layer 60 with r=r_59,
t=0 reproduces the exact initialization for blocks whose halo reaches
the surface.

The per-layer reciprocals use fused custom DVE ops based on the
geometric expansion 1/(1 -+ u) = (1 +- u)(1+u^2)(1+u^4) (error <= u^8;
here u = rs*r <= 0.21 so < 4e-6), which fits the 8-stage DVE pipeline.

Most elementwise work runs in fp16 (all magnitudes are O(1) and all
sums are of positive terms, so errors stay relative ~5e-4); the two
flux recurrences use the fp32-state tensor_tensor_scan instruction and
all outputs are written as fp32.  Measured max rel err ~4e-3 vs the
fp32 reference (tolerance 2e-2).

Emission is software-pipelined: chunk k's scan-A ops are interleaved
with chunk k-1's bulk ops so each engine's in-order queue always has
runnable work from two chunks (the engines idle through phase hand-offs
otherwise).  Input DMAs issue on the SP queue, flux output DMAs on the
Activation queue, so an output DMA waiting for compute never
head-of-line-blocks the next chunk's input loads.
"""

import numpy as np

import concourse.bass as bass
import concourse.bacc as bacc
import concourse.tile as tile
from concourse import mybir
from concourse.bass_utils import run_bass_kernel_spmd

E, L, C = 8192, 60, 48
N_CORES = 8
E_SH = E // N_CORES          # 1024 examples per core
P = 128                      # partitions per chunk
N_CHUNKS = E_SH // P         # 8 chunks per core
Lm1 = L - 1                  # 59
W = Lm1 * C                  # 2832
WL = L * C                   # 2880

K = 6                        # layers per scan-A block
B = L // K                   # 10 blocks
G = 3                        # halo layers per block
STEPS = K + G                # 9 scan steps
PADL = L + G + 1             # 64 layers incl. pseudo-layer + halo pad
BC = B * C                   # 480, scan step width

F32 = mybir.dt.float32
F16 = mybir.dt.float16
ALU = mybir.AluOpType
AFT = mybir.ActivationFunctionType

# Engine assignment for the tunable bulk ops: 'D' = DVE, 'P' = Pool (gpsimd),
# 'A' = Activation.  Chosen by greedy search against TimelineSim.
RATIO = 3          # bulk thunks emitted per scan step
FIRST_POOL = ()    # chunk-0 ramp shift (off: Pool tt is too slow to help)
SPLIT_FRAC = 36    # layers (of 59) DVE keeps for 'S'-assigned split ops
BUFS_LD = 1        # double-buffer DMA landing tiles
BULK_FIRST = False # interleave order: emit bulk thunks before each scan step
BUFS_MISC = 1      # double-buffer scan-operand/permute tiles
TAIL_LH = 16       # layer-split point for the last chunk's tail
ASSIGN = {
    "sr": "D", "ssr": "S", "c1nat": "P", "wnat": "D", "tmnat": "D",
    "sq": "P", "v": "P", "smu": "D", "b1nat": "D",
    "h2": "P", "ab": "P", "two_ip": "A",
    "b1til": "A", "c1til": "A", "wtil": "A", "tmtil": "A",
    "fu": "A", "fd": "P",
    "rp16c": "A", "t16c": "A", "s16c": "A",
}


# ---------------------------------------------------------------------------
# Custom DVE ops: fused reciprocals via the geometric expansion.
#   RECIP1M_GEO(x, y) = 1/(1 - x*y) ~ (1+u)(1+u^2)(1+u^4),  u = x*y
#   RECIP1P_GEO(x, y) = 1/(1 + x*y) ~ (1-u)(1+u^2)(1+u^4)
# ---------------------------------------------------------------------------

def _register_custom_ops():
    import concourse.dve_ops as dve_ops
    from concourse.dve_spec import (
        Spec, Src0, Src1, One, lower as dve_lower, _has_src1, sq,
    )
    from concourse.dve_uop import DveOpSpec

    def f32(x):
        return np.asarray(x, np.float32).astype(np.float32)

    def geo_recip_spec(sign):
        u = Src0 * Src1
        u2 = sq(u)
        u4 = sq(u2)
        p = (One + u) if sign > 0 else (One - u)
        body = (p * (One + u2)) * (One + u4)

        def ref(in0, in1, c0, c1, c2, _s=(1.0 if sign > 0 else -1.0)):
            u = f32(f32(in0) * f32(in1))
            u2 = f32(u * u)
            u4 = f32(u2 * u2)
            p = f32(1.0 + _s * u)
            return f32(f32(p * f32(1.0 + u2)) * f32(1.0 + u4))

        return Spec(body=body, reference=ref)

    specs = [
        ("RECIP1M_GEO", geo_recip_spec(+1)),
        ("RECIP1P_GEO", geo_recip_spec(-1)),
    ]
    by_name = {op.name: op for op in dve_ops.OPS}
    out = {}
    for name, spec in specs:
        if name in by_name:
            out[name] = by_name[name]
            continue
        row = dve_ops._CUSTOM_DVE_ROW_BASE + len(dve_ops.OPS)
        assert row < 0x20, "custom-DVE opcode rows exhausted"
        dve_ops._SUB_OPCODE_FOR_NAME[name] = row
        shas = {}
        for ver in ("v3",):
            uops = dve_lower(spec, ver=ver)
            s = DveOpSpec(name=name, opcode=row, uops=uops,
                          rd1_en=_has_src1(spec))
            shas[ver] = s.sha(ver)
        op = dve_ops.DveOp(name, spec, subdim=False, uops_sha=shas)
        dve_ops.OPS.append(op)
        dve_ops.CUSTOM_DVE_SPECS[name] = spec
        out[name] = op
    return out


_CUSTOM = _register_custom_ops()


# ---------------------------------------------------------------------------
# AP view helpers
# ---------------------------------------------------------------------------

def _lview(buf, l0, l1, rev=False):
    """[p, c, l]-ordered view of layers [l0, l1) of a [P, layers*C] buffer."""
    v = buf.rearrange("p (l c) -> p l c", c=C)[:, l0:l1]
    if rev:
        v = v[:, ::-1, :]
    return v.transpose([0, 2, 1])


def _cview(buf, t0, t1):
    """[p, c, tau] view of positions [t0, t1) of a [P, C*Lm1] scan buffer."""
    return buf.rearrange("p (c l) -> p c l", c=C)[:, :, t0:t1]


def _step_view(buf, off):
    """[P, B, C] view of layers {b*K + off} of a [P, PADL*C] buffer."""
    return buf.rearrange("p (l c) -> p l c", c=C)[:, off:off + (B - 1) * K + 1:K, :]


def _nat(buf):
    return buf.rearrange("p (l c) -> p l c", c=C)


# ---------------------------------------------------------------------------
# Per-chunk emission, split into interleavable pieces
# ---------------------------------------------------------------------------

class Chunk:
    def __init__(self, nc, pool, dram, k):
        self.nc = nc
        self.pool = pool
        self.dram = dram
        self.k = k
        self.e0 = k * P
        self.t = {}

    def emit_loads(self):
        nc, pool, e0, k = self.nc, self.pool, self.e0, self.k
        a_d, r_d, t_d, s_d = self.dram[:4]
        rp32 = pool.tile([P, WL], F32, tag="ldA", name=f"rp32_{k}", bufs=BUFS_LD)
        nc.sync.dma_start(rp32[:], r_d[e0:e0 + P].rearrange("p l c -> p (l c)"))
        t32 = pool.tile([P, WL], F32, tag="ldB", name=f"t32_{k}", bufs=BUFS_LD)
        nc.sync.dma_start(t32[:], t_d[e0:e0 + P].rearrange("p l c -> p (l c)"))
        s32 = pool.tile([P, WL], F32, tag="ldC", name=f"s32_{k}", bufs=BUFS_LD)
        nc.sync.dma_start(s32[:], s_d[e0:e0 + P].rearrange("p l c -> p (l c)"))
        self.t.update(rp32=rp32, t32=t32, s32=s32)

    def emit_load_a(self):
        # deferred: the a32 slot (bufs=1) WAR-depends on chunk k-1's
        # absorbed out-DMA, which must already be in the SP queue.
        nc, pool, e0, k = self.nc, self.pool, self.e0, self.k
        a_d = self.dram[0]
        a32 = pool.tile([P, W], F32, tag="ldD", name=f"a32_{k}", bufs=BUFS_LD)
        nc.sync.dma_start(a32[:], a_d[e0:e0 + P, :Lm1].rearrange("p l c -> p (l c)"))
        self.t.update(a32=a32)

    def emit_casts(self):
        nc, pool, k = self.nc, self.pool, self.k
        V, SC, GP = nc.vector, nc.scalar, nc.gpsimd
        rp32, t32, s32 = self.t["rp32"], self.t["t32"], self.t["s32"]
        def cast(who, dst, srcv):
            eng = ASSIGN.get(who, "A")
            if eng == "A":
                SC.copy(dst, srcv)
            elif eng == "P":
                GP.tensor_scalar(dst, srcv, 1.0, None, ALU.mult)
            else:
                V.tensor_scalar(dst, srcv, 1.0, None, ALU.mult)

        rp16 = pool.tile([P, PADL * C], F16, tag="rp16", bufs=2)
        cast("rp16c", rp16[:, :WL], rp32[:])
        cast("rp16c", rp16[:, L * C:(L + 1) * C], rp32[:, Lm1 * C:])  # pseudo-layer
        GP.memset(rp16[:, (L + 1) * C:], 0.0)
        t2p = pool.tile([P, PADL * C], F16, tag="t2p")
        SC.square(t2p[:, :WL], t32[:])     # before t16/s16: unblocks the scan
        GP.memset(t2p[:, L * C:], 0.0)
        t16 = pool.tile([P, WL], F16, tag="t16", bufs=2)
        cast("t16c", t16[:], t32[:])
        s16 = pool.tile([P, WL], F16, tag="s16", bufs=2)
        cast("s16c", s16[:], s32[:])
        ret_rs = pool.tile([P, PADL * C], F16, tag="ret_rs", bufs=2)
        ret_id = pool.tile([P, PADL * C], F16, tag="ret_id", bufs=2)
        scr = pool.tile([P, 2 * 2 * BC], F16, tag="scr")
        self.t.update(rp16=rp16, t16=t16, s16=s16, t2p=t2p,
                      ret_rs=ret_rs, ret_id=ret_id, scr=scr)

    def scan_thunks(self):
        nc = self.nc
        V = nc.vector

        def step(j):
            rp16, t2p = self.t["rp16"], self.t["t2p"]
            ret_rs, ret_id, scr = self.t["ret_rs"], self.t["ret_id"], self.t["scr"]
            off = K - 1 + G - j
            carry = (_step_view(rp16[:], K + G) if j == 0
                     else _step_view(ret_rs[:], off + 1))
            r_l = _step_view(rp16[:], off)
            t2_l = _step_view(t2p[:], off)
            base = (j % 2) * 2 * BC
            P_s = scr[:, base:base + BC].rearrange("p (b c) -> p b c", c=C)
            num_s = scr[:, base + BC:base + 2 * BC].rearrange("p (b c) -> p b c", c=C)
            id_s = _step_view(ret_id[:], off)
            GP = nc.gpsimd
            V._custom_dve(_CUSTOM["RECIP1M_GEO"], out=id_s, in0=carry, in1=r_l)
            if ASSIGN.get("scanP", "D") == "D":
                V.tensor_tensor(P_s, carry, t2_l, ALU.mult)
            else:
                GP.scalar_tensor_tensor(P_s, carry, 1.0, t2_l, ALU.mult, ALU.mult)
            if ASSIGN.get("scanNum", "D") == "D":
                V.tensor_tensor(num_s, P_s, r_l, ALU.add)
            else:
                GP.scalar_tensor_tensor(num_s, P_s, 0.0, r_l, ALU.add, ALU.add)
            if ASSIGN.get("scanRs", "D") == "D":
                V.tensor_tensor(_step_view(ret_rs[:], off), num_s, id_s, ALU.mult)
            else:
                GP.scalar_tensor_tensor(_step_view(ret_rs[:], off), num_s, 1.0,
                                        id_s, ALU.mult, ALU.mult)

        return [lambda j=j: step(j) for j in range(STEPS)]

    def bulk_thunks(self, first=False, last=False):
        nc, pool, k, e0 = self.nc, self.pool, self.k, self.e0
        fu_d, fd_d, ab_d = self.dram[4:]
        V, GP, SC = nc.vector, nc.gpsimd, nc.scalar
        T = self.t
        th = []

        def tt(who, dst, a, b, op):
            # gpsimd supports only plain TensorTensor of the 2-tensor ops
            # (scalar_tensor_tensor and tensor_tensor_scan fail the Pool
            # engine check in the real backend)
            eng = ASSIGN[who]
            if first and who in FIRST_POOL:
                eng = "S"
            if eng == "P":
                GP.tensor_tensor(dst, a, b, op)
            elif eng == "S":
                # split: DVE takes the front, Pool the (cheaper) back piece
                cut = SPLIT_FRAC * C
                V.tensor_tensor(dst[:, :cut], a[:, :cut], b[:, :cut], op)
                GP.tensor_tensor(dst[:, cut:], a[:, cut:], b[:, cut:], op)
            else:
                V.tensor_tensor(dst, a, b, op)

        def perm(who, dst, srcv):
            if ASSIGN[who] == "A":
                SC.copy(dst, srcv)
            elif ASSIGN[who] == "P":
                GP.tensor_copy(dst, srcv)
            else:
                V.tensor_scalar(dst, srcv, 1.0, None, ALU.mult)

        # emission order = per-engine queue order; put ops whose inputs are
        # ready at round start (they only need scan A's outputs) first, so
        # ACT's permutes and POOL never head-of-line-wait on late producers.
        def t_sr():
            sr = pool.tile([P, W], F16, tag="sr_ssr", bufs=BUFS_MISC)
            tt("sr", sr[:], T["s16"][:, C:], T["rp16"][:, :W], ALU.mult)
            T["sr"] = sr
        th.append(t_sr)

        def t_ssr():
            tt("ssr", T["sr"][:], T["s16"][:, :W], T["sr"][:], ALU.add)
        th.append(t_ssr)

        def t_c1nat():
            tt("c1nat", T["sr"][:], T["sr"][:], T["ret_id"][:, :W], ALU.mult)
        th.append(t_c1nat)

        def t_wnat():
            wnat = pool.tile([P, W], F16, tag="wnat_futil", name=f"wnat_{k}")
            tt("wnat", wnat[:], T["t16"][:, :W], T["ret_id"][:, :W], ALU.mult)
            T["wnat"] = wnat
        th.append(t_wnat)

        def t_c1til():
            c1til = pool.tile([P, W], F16, tag="c1til", bufs=BUFS_MISC)
            perm("c1til", _cview(c1til[:], 0, Lm1), _lview(T["sr"][:], 0, Lm1))
            T["c1til"] = c1til
        th.append(t_c1til)

        def t_wtil():
            wtil = pool.tile([P, W], F16, tag="wtil", bufs=BUFS_MISC)
            GP.memset(wtil[:, 0:W:Lm1], 0.0)
            perm("wtil", _cview(wtil[:], 1, Lm1), _lview(T["wnat"][:], 1, Lm1, rev=True))
            T["wtil"] = wtil
        th.append(t_wtil)

        def t_ip():
            rp16, ret_rs = T["rp16"], T["ret_rs"]
            ip = pool.tile([P, W], F16, tag="ip", bufs=BUFS_MISC)
            V._custom_dve(_CUSTOM["RECIP1P_GEO"], out=ip[:],
                          in0=ret_rs[:, C:L * C], in1=rp16[:, :W])
            T["ip"] = ip
        th.append(t_ip)

        def t_q():
            q = pool.tile([P, W], F16, tag="q_v", bufs=BUFS_MISC)
            if ASSIGN.get("q") == "S":
                cut = SPLIT_FRAC * C
                V.tensor_tensor(q[:, :cut], T["ret_rs"][:, C:C + cut], T["ip"][:, :cut], ALU.mult)
                GP.tensor_tensor(q[:, cut:], T["ret_rs"][:, C + cut:L * C], T["ip"][:, cut:], ALU.mult)
            else:
                V.tensor_tensor(q[:], T["ret_rs"][:, C:L * C], T["ip"][:], ALU.mult)
            T["q"] = q
        th.append(t_q)

        def t_twoip():
            two_ip = pool.tile([P, W], F16, tag="two_ip")
            if ASSIGN.get("two_ip", "D") == "A":
                SC.activation(two_ip[:], T["ip"][:], AFT.Copy, bias=2.0, scale=-1.0)
            else:
                V.tensor_scalar(two_ip[:], T["ip"][:], -1.0, 2.0, ALU.mult, ALU.add)
            T["two_ip"] = two_ip
        th.append(t_twoip)

        def t_smu():
            smu = pool.tile([P, W], F16, tag="smu", bufs=BUFS_MISC)
            tt("smu", smu[:], T["two_ip"][:], T["s16"][:, C:], ALU.mult)
            T["smu"] = smu
        th.append(t_smu)

        def t_sq():
            sq = pool.tile([P, W], F16, tag="sq_h1", name=f"sq_{k}")
            tt("sq", sq[:], T["s16"][:, :W], T["q"][:], ALU.mult)
            T["sq"] = sq
        th.append(t_sq)

        def t_tmnat():
            tmnat = pool.tile([P, W], F16, tag="tmnat", bufs=BUFS_MISC)
            tt("tmnat", tmnat[:], T["t16"][:, :W], T["ip"][:], ALU.mult)
            T["tmnat"] = tmnat
        th.append(t_tmnat)

        def t_tmtil():
            tmtil = pool.tile([P, W], F16, tag="tmtil", bufs=BUFS_MISC)
            GP.memset(tmtil[:, 0:W:Lm1], 0.0)
            perm("tmtil", _cview(tmtil[:], 1, Lm1), _lview(T["tmnat"][:], 0, Lm1 - 1))
            T["tmtil"] = tmtil
        th.append(t_tmtil)

        def t_v():
            tt("v", T["q"][:], T["t16"][:, :W], T["q"][:], ALU.mult)
        th.append(t_v)

        def t_b1nat():
            tt("b1nat", T["smu"][:], T["smu"][:], T["sq"][:], ALU.add)
        th.append(t_b1nat)

        def t_b1til():
            b1til = pool.tile([P, W], F16, tag="b1til", bufs=BUFS_MISC)
            perm("b1til", _cview(b1til[:], 0, Lm1), _lview(T["smu"][:], 0, Lm1, rev=True))
            T["b1til"] = b1til
        th.append(t_b1til)

        # --- flux scans + absorbed tail ---
        # For the last chunk there is no next scan to overlap with, so this
        # serial chain IS the program tail: split every op into channel
        # halves and run them on Pool and DVE concurrently.
        Wh = W // 2          # first 24 of 48 channels in (c, tau) scan layout
        Ch = C // 2

        def chalves(buf_nat3):
            return buf_nat3[:, :, :Ch], buf_nat3[:, :, Ch:]

        def t_scanB():
            # tensor_tensor_scan only exists on the DVE engine
            futil = pool.tile([P, W], F16, tag="wnat_futil", name=f"futil_{k}")
            V.tensor_tensor_scan(futil[:], T["wtil"][:], T["b1til"][:], 0.0,
                                 ALU.mult, ALU.add)
            T["futil"] = futil

        def t_scanC():
            fdtil = pool.tile([P, W], F16, tag="two_ip", name=f"fdtil_{k}")
            V.tensor_tensor_scan(fdtil[:], T["tmtil"][:], T["c1til"][:], 0.0,
                                 ALU.mult, ALU.add)
            T["fdtil"] = fdtil
        th.append(t_scanC)
        th.append(t_scanB)

        Lh = TAIL_LH         # layer-split point for the tail ops/DMAs
        WA, WB = Lh * C, W - Lh * C

        def fu_view(lhalf=None):
            # futil is (c, tau) with tau = 58 - l
            if lhalf is None:
                v = _cview(T["futil"][:], 0, Lm1)[:, :, ::-1]
            elif lhalf == 0:     # layers 0..Lh-1  -> tau 58-Lh+1..58
                v = _cview(T["futil"][:], Lm1 - Lh, Lm1)[:, :, ::-1]
            else:                # layers Lh..58   -> tau 0..58-Lh
                v = _cview(T["futil"][:], 0, Lm1 - Lh)[:, :, ::-1]
            return v.transpose([0, 2, 1])

        def fd_view(lhalf=None):
            if lhalf is None:
                v = _cview(T["fdtil"][:], 0, Lm1)
            elif lhalf == 0:
                v = _cview(T["fdtil"][:], 0, Lh)
            else:
                v = _cview(T["fdtil"][:], Lh, Lm1)
            return v.transpose([0, 2, 1])

        def lhalves(buf2d):
            return (buf2d[:, :WA].rearrange("p (l c) -> p l c", c=C),
                    buf2d[:, WA:].rearrange("p (l c) -> p l c", c=C))

        def t_fu():
            fu_nat = pool.tile([P, W], F32, tag="fu_nat", bufs=BUFS_MISC)
            T["fu_nat"] = fu_nat
            dram_v = fu_d[e0:e0 + P].rearrange("p l c -> p (l c)")
            if last:
                h0, h1_ = lhalves(fu_nat[:])
                SC.copy(h0, fu_view(0))
                nc.scalar.dma_start(dram_v[:, :WA], fu_nat[:, :WA])
                V.tensor_scalar(h1_, fu_view(1), 1.0, None, ALU.mult)
                nc.scalar.dma_start(dram_v[:, WA:], fu_nat[:, WA:])
                return
            if ASSIGN["fu"] == "A":
                SC.copy(_nat(fu_nat[:]), fu_view())
            elif ASSIGN["fu"] == "P":
                GP.tensor_copy(_nat(fu_nat[:]), fu_view())
            else:
                V.tensor_scalar(_nat(fu_nat[:]), fu_view(), 1.0, None, ALU.mult)
            nc.scalar.dma_start(dram_v, fu_nat[:])
        th.append(t_fu)

        def t_fd():
            fd_nat = pool.tile([P, W], F32, tag="fd_nat", bufs=BUFS_MISC)
            T["fd_nat"] = fd_nat
            dram_v = fd_d[e0:e0 + P].rearrange("p l c -> p (l c)")
            if last:
                h0, h1_ = lhalves(fd_nat[:])
                SC.copy(h0, fd_view(0))
                nc.scalar.dma_start(dram_v[:, :WA], fd_nat[:, :WA])
                V.tensor_scalar(h1_, fd_view(1), 1.0, None, ALU.mult)
                nc.scalar.dma_start(dram_v[:, WA:], fd_nat[:, WA:])
                return
            if ASSIGN["fd"] == "A":
                SC.copy(_nat(fd_nat[:]), fd_view())
            elif ASSIGN["fd"] == "P":
                GP.tensor_copy(_nat(fd_nat[:]), fd_view())
            else:
                V.tensor_scalar(_nat(fd_nat[:]), fd_view(), 1.0, None, ALU.mult)
            nc.scalar.dma_start(dram_v, fd_nat[:])
        th.append(t_fd)

        def t_h1():
            h1 = pool.tile([P, W], F16, tag="sq_h1", name=f"h1_{k}")
            V.scalar_tensor_tensor(_nat(h1[:]), _nat(T["q"][:]), 1.0,
                                   fd_view(), ALU.add, ALU.mult)
            T["h1"] = h1
        th.append(t_h1)

        def t_h2():
            if last:
                hh0, hh1 = lhalves(T["h1"][:])
                GP.tensor_tensor(hh0, hh0, fu_view(0), ALU.add)
                V.scalar_tensor_tensor(hh1, hh1, 0.0, fu_view(1), ALU.add, ALU.add)
            elif ASSIGN["h2"] == "P":
                GP.tensor_tensor(_nat(T["h1"][:]), _nat(T["h1"][:]),
                                 fu_view(), ALU.add)
            else:
                V.scalar_tensor_tensor(_nat(T["h1"][:]), _nat(T["h1"][:]), 0.0,
                                       fu_view(), ALU.add, ALU.add)
        th.append(t_h2)

        def t_ab():
            dram_v = ab_d[e0:e0 + P].rearrange("p l c -> p (l c)")
            if last:
                ah0, ah1 = lhalves(T["a32"][:])
                hh0, hh1 = lhalves(T["h1"][:])
                GP.tensor_tensor(ah0, ah0, hh0, ALU.mult)
                nc.sync.dma_start(dram_v[:, :WA], T["a32"][:, :WA])
                V.tensor_tensor(ah1, ah1, hh1, ALU.mult)
                nc.sync.dma_start(dram_v[:, WA:], T["a32"][:, WA:])
                return
            tt("ab", T["a32"][:], T["a32"][:], T["h1"][:], ALU.mult)
            nc.sync.dma_start(dram_v, T["a32"][:])
        th.append(t_ab)

        return th


def build_bass():
    nc = bacc.Bacc("TRN2", target_bir_lowering=False, debug=False)
    a_d = nc.dram_tensor("a", [E_SH, L, C], F32, kind="ExternalInput").ap()
    r_d = nc.dram_tensor("r", [E_SH, L, C], F32, kind="ExternalInput").ap()
    t_d = nc.dram_tensor("t", [E_SH, L, C], F32, kind="ExternalInput").ap()
    s_d = nc.dram_tensor("s", [E_SH, L, C], F32, kind="ExternalInput").ap()
    fu_d = nc.dram_tensor("flux_up", [E_SH, Lm1, C], F32, kind="ExternalOutput").ap()
    fd_d = nc.dram_tensor("flux_down", [E_SH, Lm1, C], F32, kind="ExternalOutput").ap()
    ab_d = nc.dram_tensor("absorbed", [E_SH, Lm1, C], F32, kind="ExternalOutput").ap()
    dram = (a_d, r_d, t_d, s_d, fu_d, fd_d, ab_d)

    with tile.TileContext(nc) as tc:
        with tc.tile_pool(name="pool", bufs=1) as pool:
            chunks = [Chunk(nc, pool, dram, k) for k in range(N_CHUNKS)]
            prev_bulk = []
            for k in range(N_CHUNKS):
                ch = chunks[k]
                ch.emit_loads()
                ch.emit_casts()
                scan = ch.scan_thunks()
                # interleave this chunk's scan with the previous chunk's bulk
                si, bi = 0, 0
                while si < len(scan) or bi < len(prev_bulk):
                    if BULK_FIRST:
                        for _ in range(RATIO):
                            if bi < len(prev_bulk):
                                prev_bulk[bi]()
                                bi += 1
                        if si < len(scan):
                            scan[si]()
                            si += 1
                    else:
                        if si < len(scan):
                            scan[si]()
                            si += 1
                        for _ in range(RATIO):
                            if bi < len(prev_bulk):
                                prev_bulk[bi]()
                                bi += 1
                ch.emit_load_a()
                prev_bulk = ch.bulk_thunks(first=(k == 0),
                                           last=(k == N_CHUNKS - 1))
            for th in prev_bulk:
                th()
    nc.compile()
    return nc


_NC_CACHE = None


def kernel(a, r, t, s):
    global _NC_CACHE
    if _NC_CACHE is None:
        _NC_CACHE = build_bass()
    nc = _NC_CACHE
    in_maps = []
    for i in range(N_CORES):
        sl = slice(i * E_SH, (i + 1) * E_SH)
        in_maps.append({
            "a": np.ascontiguousarray(a[sl]),
            "r": np.ascontiguousarray(r[sl]),
            "t": np.ascontiguousarray(t[sl]),
            "s": np.ascontiguousarray(s[sl]),
        })
    res = run_bass_kernel_spmd(nc, in_maps, core_ids=list(range(N_CORES)))
    fu = np.concatenate([res.results[i]["flux_up"] for i in range(N_CORES)], axis=0)
    fd = np.concatenate([res.results[i]["flux_down"] for i in range(N_CORES)], axis=0)
    ab = np.concatenate([res.results[i]["absorbed"] for i in range(N_CORES)], axis=0)
    return fu, fd, ab
